# revision 15
# baseline (speedup 1.0000x reference)
"""Causal self-attention (B=4, T=2048, D=1024, H=16, hd=64) on 8 trn2 NeuronCores.

Sharding: data parallel over batch (4) x tensor parallel over heads (2 groups
of 8). Core c handles batch c//2 and heads (c%2)*8 .. (c%2)*8+8.
Wq/Wk/Wv are column-parallel by head group, Wo row-parallel; the pair of
cores sharing a batch produce partial outputs that are summed on the host.

On-device layout (per core) is fully "transposed": projections produce
Q^T, K^T [512, 2048] and V [2048, 512], scores are computed as
S^T = K Q^T (j=key on partitions, i=query on free dim), softmax uses
exp without max subtraction (scores are O(6) here), the denominator
comes for free from a ones-column appended to V, and attention output
O^T [hd, T] feeds the row-parallel out-projection directly as lhsT.

Head pairs share one [128, 1024] exp; their S^T matmuls row-pack onto
the PE concurrently (partition offsets 0/64). The per-chunk emission is
software-pipelined (S of tile jt+1 ahead of AV of tile jt in the PE
stream) so the PE never waits on the scalar engine's exp.
"""

import contextlib
import ctypes
import sys
import types

import numpy as np

B, T, D = 4, 2048, 1024
H_TOT, HD = 16, 64
SCALE = HD ** -0.5
P = 128
NH = 8            # heads per core
QD = NH * HD      # 512, projected dim per core
KT = D // P       # 8 contraction tiles for projections
MT = QD // P      # 4 qdim tiles
TT = T // P       # 16 token tiles
ACH = 512         # phase-A1 token chunk (Q/K); PSUM bank caps matmul N at 512
NACH = T // ACH   # 4
ICH = 512         # attention query chunk
NIC = T // ICH    # 4

_PROGRAM = None  # compiled program cache — build once per process


def _install_ntff_hook():
    """antenv.axon_hooks is missing in this image; recreate it so
    run_bass_kernel_spmd(trace=True) can profile. Harmless if unused."""
    if "antenv.axon_hooks" in sys.modules:
        return
    try:
        import antenv
    except ImportError:
        return
    mod = types.ModuleType("antenv.axon_hooks")
    _hook = [None]
    mod.set_axon_ntff_profile_hook = lambda h: _hook.__setitem__(0, h)
    mod.get_axon_ntff_profile_hook = lambda: _hook[0]
    antenv.axon_hooks = mod
    sys.modules["antenv.axon_hooks"] = mod
    try:
        lib = ctypes.CDLL("/opt/axon/libaxon_pjrt.so")
        if not hasattr(lib, "axon_start_nrt_profile"):
            return
        lib.axon_start_nrt_profile.argtypes = [
            ctypes.POINTER(ctypes.c_int64), ctypes.c_size_t]
        lib.axon_start_nrt_profile.restype = ctypes.c_int64
        lib.axon_stop_nrt_profile.argtypes = [ctypes.c_char_p]
        lib.axon_stop_nrt_profile.restype = ctypes.c_int64

        @contextlib.contextmanager
        def _hookfn(output_dir, device_ids):
            import jax
            jax.devices()
            if device_ids:
                ids = (ctypes.c_int64 * len(device_ids))(*device_ids)
                rc = lib.axon_start_nrt_profile(ids, len(device_ids))
            else:
                rc = lib.axon_start_nrt_profile(None, 0)
            if rc != 0:
                raise RuntimeError(f"axon_start_nrt_profile rc={rc}")
            try:
                yield
            finally:
                n = lib.axon_stop_nrt_profile(str(output_dir).encode())
                print(f"profile: {n} file(s) written to {output_dir}")

        mod.set_axon_ntff_profile_hook(_hookfn)
    except OSError:
        pass


def _build_program():
    from contextlib import ExitStack

    import concourse.tile as tile
    from concourse import bacc, mybir

    F32 = mybir.dt.float32
    BF16 = mybir.dt.bfloat16
    AF = mybir.ActivationFunctionType
    ALU = mybir.AluOpType

    nc = bacc.Bacc("TRN2", target_bir_lowering=False, debug=False,
                   num_devices=8)

    # all tensor inputs arrive pre-arranged in SBUF layout [128, k, n]
    # (host does the transpose) so every DMA is long contiguous runs
    xT_d = nc.dram_tensor("xT", [P, KT * T], BF16, kind="ExternalInput").ap()
    wq_d = nc.dram_tensor("wq", [P, KT * QD], BF16, kind="ExternalInput").ap()
    wk_d = nc.dram_tensor("wk", [P, KT * QD], BF16, kind="ExternalInput").ap()
    wv_d = nc.dram_tensor("wv", [P, KT * QD], BF16, kind="ExternalInput").ap()
    wo_d = nc.dram_tensor("wo", [P, MT * D], BF16, kind="ExternalInput").ap()
    bq_d = nc.dram_tensor("bq", [P, MT], F32, kind="ExternalInput").ap()
    bk_d = nc.dram_tensor("bk", [P, MT], F32, kind="ExternalInput").ap()
    bv_d = nc.dram_tensor("bv", [1, QD], BF16, kind="ExternalInput").ap()
    msk_d = nc.dram_tensor("msk", [P, P], BF16, kind="ExternalInput").ap()
    ones_d = nc.dram_tensor("ones", [P, P], BF16, kind="ExternalInput").ap()
    out_d = nc.dram_tensor("out", [T, D], F32, kind="ExternalOutput").ap()

    xT_k = xT_d.rearrange("p (k t) -> p k t", k=KT)      # [128, 8, 2048]
    wq_k = wq_d.rearrange("p (k m) -> p k m", k=KT)      # [128, 8, 512]
    wk_k = wk_d.rearrange("p (k m) -> p k m", k=KT)
    wv_k = wv_d.rearrange("p (k m) -> p k m", k=KT)
    wo_k = wo_d.rearrange("p (k e) -> p k e", k=MT)      # [128, 4, 1024]

    with tile.TileContext(nc) as tc, ExitStack() as ctx:
        persist = ctx.enter_context(tc.tile_pool(name="persist", bufs=1))

        qt = [persist.tile([P, T], BF16, name=f"qt{i}") for i in range(MT)]
        kt_ = [persist.tile([P, T], BF16, name=f"kt{i}") for i in range(MT)]
        v3 = [persist.tile([P, NH, HD + 1], BF16, name=f"v3_{i}")
              for i in range(TT)]
        at = [persist.tile([P, T], BF16, name=f"at{i}") for i in range(MT)]
        xt_all = persist.tile([P, KT, T], BF16, name="xt")

        bq_sb = persist.tile([P, MT], F32, name="bq")
        bk_sb = persist.tile([P, MT], F32, name="bk")
        bv_sb = persist.tile([1, QD], BF16, name="bv")
        onesrow = persist.tile([1, P], BF16, name="onesrow")
        tri_sb = persist.tile([P, P], BF16, name="tri")
        wv_sb = persist.tile([P, KT, QD], BF16, name="wv")
        wo_sb = persist.tile([P, MT, D], BF16, name="wo")

        nc.sync.dma_start(xt_all[:, :, 0:T // 2], xT_k[:, :, 0:T // 2])
        nc.sync.dma_start(bq_sb[:], bq_d)
        nc.sync.dma_start(bk_sb[:], bk_d)
        nc.sync.dma_start(xt_all[:, :, T // 2:T], xT_k[:, :, T // 2:T])
        nc.sync.dma_start(bv_sb[:], bv_d)
        nc.sync.dma_start(onesrow[:], ones_d[0:1, :])
        nc.sync.dma_start(tri_sb[:], msk_d)
        nc.sync.dma_start(wv_sb[:], wv_k)
        nc.sync.dma_start(wo_sb[:], wo_k)
        for tt in range(TT):
            nc.vector.memset(v3[tt][:, :, HD:HD + 1], 1.0)

        # ---- phase A1: Q^T, K^T projections --------------------------------
        with tc.tile_pool(name="wqk", bufs=1) as wp, \
             tc.tile_pool(name="pjps1", bufs=1, space="PSUM") as pjp:
            wq_sb = wp.tile([P, KT, QD], BF16, name="wq")
            wk_sb = wp.tile([P, KT, QD], BF16, name="wk")
            nc.sync.dma_start(wq_sb[:], wq_k)
            nc.sync.dma_start(wk_sb[:], wk_k)
            # chunk-inner so each weight tile is loaded into the PE once
            # and reused for all four 512-column chunks
            for mt in range(MT):
                for w_sb, dst, b_sb in ((wq_sb, qt, bq_sb), (wk_sb, kt_, bk_sb)):
                    ps = [pjp.tile([P, ACH], F32, name="pj", bufs=8)
                          for _ in range(NACH)]
                    for k in range(KT):
                        for nch in range(NACH):
                            nc.tensor.matmul(
                                ps[nch][:], w_sb[:, k, mt * P:(mt + 1) * P],
                                xt_all[:, k, nch * ACH:(nch + 1) * ACH],
                                start=(k == 0), stop=(k == KT - 1))
                    for nch in range(NACH):
                        csl = slice(nch * ACH, (nch + 1) * ACH)
                        nc.vector.tensor_scalar_add(dst[mt][:, csl],
                                                    ps[nch][:],
                                                    b_sb[:, mt:mt + 1])

        # ---- phases A2/B/C interleaved per query chunk ---------------------
        with tc.tile_pool(name="attnsb", bufs=1) as ap_, \
             tc.tile_pool(name="obp", bufs=3) as obp, \
             tc.tile_pool(name="attnps", bufs=1, space="PSUM") as sp:

            def emit_v_tile(tt):
                psv = sp.tile([P, QD], F32, name="misc", bufs=1)
                for k in range(KT):
                    nc.tensor.matmul(
                        psv[:], xt_all[:, k, tt * P:(tt + 1) * P],
                        wv_sb[:, k, :], start=(k == 0), stop=False)
                nc.tensor.matmul(psv[:], onesrow[0:1, :], bv_sb[0:1, :],
                                 start=False, stop=True)
                nc.vector.tensor_copy(
                    v3[tt][:, :, 0:HD],
                    psv[:].rearrange("p (h d) -> p h d", d=HD))

            def emit_attn_chunk(ic):
                """Attention for query chunk ic, as 4 head-pairs, with the
                PE stream software-pipelined: S of j-tile jt+1 is emitted
                before AV of j-tile jt so exp latency is hidden."""
                isl = slice(ic * ICH, (ic + 1) * ICH)
                njt = 4 * ic + 4
                pending = []
                for hp in range(MT):
                    opsA = sp.tile([HD + 1, ICH], F32, name="opsum", bufs=3)
                    opsB = sp.tile([HD + 1, ICH], F32, name="opsum", bufs=3)
                    s2s, e2s = {}, {}

                    def emit_s(jt):
                        s2 = sp.tile([P, 2 * ICH], F32, name="spsum", bufs=2)
                        jsl = slice(jt * P, (jt + 1) * P)
                        nc.tensor.matmul(s2[:, 0:ICH], kt_[hp][0:HD, jsl],
                                         qt[hp][0:HD, isl],
                                         start=True, stop=True)
                        nc.tensor.matmul(s2[:, ICH:2 * ICH],
                                         kt_[hp][HD:P, jsl],
                                         qt[hp][HD:P, isl],
                                         start=True, stop=True)
                        s2s[jt] = s2

                    def emit_exp(jt):
                        e2 = ap_.tile([P, 2 * ICH], BF16, name="e", bufs=3)
                        nc.scalar.activation(e2[:], s2s.pop(jt)[:], AF.Exp)
                        kdiag = jt - 4 * ic
                        if kdiag >= 0:
                            # zero the diagonal block's upper triangle
                            c0 = kdiag * P
                            for half in range(2):
                                o = half * ICH + c0
                                nc.vector.tensor_tensor(
                                    e2[:, o:o + P], e2[:, o:o + P],
                                    tri_sb[:], op=ALU.mult)
                        e2s[jt] = e2

                    def emit_av(jt):
                        # columns left of the diagonal block are causally
                        # invalid — restrict the accumulation to the valid
                        # column range instead of zeroing them
                        kdiag = jt - 4 * ic
                        c0 = max(kdiag, 0) * P
                        e2 = e2s.pop(jt)
                        nc.tensor.matmul(opsA[:, c0:], v3[jt][:, 2 * hp, :],
                                         e2[:, c0:ICH],
                                         start=(jt == 0),
                                         stop=(jt == njt - 1))
                        nc.tensor.matmul(opsB[:, c0:],
                                         v3[jt][:, 2 * hp + 1, :],
                                         e2[:, ICH + c0:2 * ICH],
                                         start=(jt == 0),
                                         stop=(jt == njt - 1))

                    emit_s(0)
                    for jt in range(1, njt):
                        emit_s(jt)
                        if jt == 2 and pending:
                            pending.pop()()
                        emit_exp(jt - 1)
                        emit_av(jt - 1)
                    emit_exp(njt - 1)
                    emit_av(njt - 1)

                    if ic == 0 and pending:
                        pending.pop()()

                    def normalize(hp=hp, opsA=opsA, opsB=opsB):
                        # normalize straight out of PSUM: in0 is PSUM so the
                        # SBUF base-partition pairing rule doesn't apply
                        for half, ops in ((0, opsA), (1, opsB)):
                            po = half * HD
                            dn = ap_.tile([1, ICH], F32, name="dn", bufs=4)
                            nc.vector.tensor_copy(dn[:], ops[HD:HD + 1, :])
                            recip = ap_.tile([1, ICH], F32, name="recip",
                                             bufs=4)
                            nc.vector.reciprocal_approx_fast(recip[:], dn[:])
                            rb = ap_.tile([HD, ICH], F32, name="rb", bufs=4)
                            nc.gpsimd.partition_broadcast(rb[:], recip[:])
                            nc.vector.tensor_tensor(
                                at[hp][po:po + HD, isl], ops[0:HD, :], rb[:],
                                op=ALU.mult)

                    pending.append(normalize)

                while pending:
                    pending.pop()()

            def emit_out_chunk(ic):
                for mt in range(4 * ic, 4 * ic + 4):
                    for nch2 in range(2):
                        pso = sp.tile([P, 512], F32, name="misc", bufs=1)
                        for k in range(MT):
                            nc.tensor.matmul(
                                pso[:], at[k][:, mt * P:(mt + 1) * P],
                                wo_sb[:, k, nch2 * 512:(nch2 + 1) * 512],
                                start=(k == 0), stop=(k == MT - 1))
                        ob = obp.tile([P, 512], F32, name="ob")
                        nc.vector.tensor_copy(ob[:], pso[:])
                        nc.sync.dma_start(
                            out_d[mt * P:(mt + 1) * P,
                                  nch2 * 512:(nch2 + 1) * 512], ob[:])

            for ic in range(NIC):
                for tt in range(4 * ic, 4 * ic + 4):
                    emit_v_tile(tt)
                emit_attn_chunk(ic)
                emit_out_chunk(ic)

    nc.compile()
    return nc


def _get_program():
    global _PROGRAM
    if _PROGRAM is None:
        _install_ntff_hook()
        _PROGRAM = _build_program()
    return _PROGRAM


def _make_masks():
    """Multiplicative upper-triangle zero mask [128, 128] for the diagonal
    128x128 block of each S^T tile: entry (j, i) = 1 if j <= i else 0."""
    j = np.arange(P)[:, None]
    i = np.arange(P)[None, :]
    return (j <= i).astype(np.float32)


def make_in_maps(x, Wq, bq, Wk, bk, Wv, bv, Wo, bo):
    import ml_dtypes
    bf16 = ml_dtypes.bfloat16

    def sbl(a, k):
        """[k*128, n] -> SBUF layout [128, k*n] (partition-major runs)."""
        n = a.shape[1]
        return np.ascontiguousarray(
            a.reshape(k, P, n).transpose(1, 0, 2).reshape(P, k * n)
        ).astype(bf16)

    masks = _make_masks()
    ones = np.ones((P, P), dtype=np.float32)
    in_maps = []
    for c in range(8):
        b, hg = c // 2, c % 2
        sl = slice(hg * QD, (hg + 1) * QD)
        in_maps.append({
            "xT": sbl(np.ascontiguousarray(x[b].T), KT),
            "wq": sbl(Wq[:, sl] * SCALE, KT),
            "wk": sbl(Wk[:, sl], KT),
            "wv": sbl(Wv[:, sl], KT),
            "wo": sbl(Wo[sl, :], MT),
            "bq": np.ascontiguousarray((bq[sl] * SCALE).reshape(MT, P).T),
            "bk": np.ascontiguousarray(bk[sl].reshape(MT, P).T),
            "bv": bv[sl].reshape(1, QD).astype(bf16),
            "msk": masks.astype(bf16),
            "ones": ones.astype(bf16),
        })
    return in_maps


def run(inputs, trace=False):
    from concourse.bass_utils import run_bass_kernel_spmd

    nc = _get_program()
    in_maps = make_in_maps(**inputs)
    res = run_bass_kernel_spmd(nc, in_maps, list(range(8)), trace=trace)
    bo = inputs["bo"]
    out = np.empty((B, T, D), dtype=np.float32)
    for b in range(B):
        out[b] = res.results[2 * b]["out"] + res.results[2 * b + 1]["out"] + bo
    return out, res


def kernel(**inputs):
    inputs = {k: np.asarray(v) for k, v in inputs.items()}
    out, _ = run(inputs)
    return out


# revision 16
# speedup vs baseline: 1.1559x; 1.1559x over previous
"""Causal self-attention (B=4, T=2048, D=1024, H=16, hd=64) on 8 trn2 NeuronCores.

Sharding: data parallel over batch (4) x tensor parallel over heads (2 groups
of 8). Core c handles batch c//2 and heads (c%2)*8 .. (c%2)*8+8.
Wq/Wk/Wv are column-parallel by head group, Wo row-parallel; the pair of
cores sharing a batch produce partial outputs that are summed on the host.

On-device layout (per core) is fully "transposed": projections produce
Q^T, K^T [512, 2048] and V [2048, 512], scores are computed as
S^T = K Q^T (j=key on partitions, i=query on free dim), softmax uses
exp without max subtraction (scores are O(6) here), the denominator
comes for free from a ones-column appended to V, and attention output
O^T [hd, T] feeds the row-parallel out-projection directly as lhsT.

Head pairs share one [128, 1024] exp; their S^T matmuls row-pack onto
the PE concurrently (partition offsets 0/64). The per-chunk emission is
software-pipelined (S of tile jt+1 ahead of AV of tile jt in the PE
stream) so the PE never waits on the scalar engine's exp.
"""

import contextlib
import ctypes
import sys
import types

import numpy as np

B, T, D = 4, 2048, 1024
H_TOT, HD = 16, 64
SCALE = HD ** -0.5
P = 128
NH = 8            # heads per core
QD = NH * HD      # 512, projected dim per core
KT = D // P       # 8 contraction tiles for projections
MT = QD // P      # 4 qdim tiles
TT = T // P       # 16 token tiles
ACH = 512         # phase-A1 token chunk (Q/K); PSUM bank caps matmul N at 512
NACH = T // ACH   # 4
ICH = 512         # attention query chunk
NIC = T // ICH    # 4

_PROGRAM = None  # compiled program cache — build once per process


def _install_ntff_hook():
    """antenv.axon_hooks is missing in this image; recreate it so
    run_bass_kernel_spmd(trace=True) can profile. Harmless if unused."""
    if "antenv.axon_hooks" in sys.modules:
        return
    try:
        import antenv
    except ImportError:
        return
    mod = types.ModuleType("antenv.axon_hooks")
    _hook = [None]
    mod.set_axon_ntff_profile_hook = lambda h: _hook.__setitem__(0, h)
    mod.get_axon_ntff_profile_hook = lambda: _hook[0]
    antenv.axon_hooks = mod
    sys.modules["antenv.axon_hooks"] = mod
    try:
        lib = ctypes.CDLL("/opt/axon/libaxon_pjrt.so")
        if not hasattr(lib, "axon_start_nrt_profile"):
            return
        lib.axon_start_nrt_profile.argtypes = [
            ctypes.POINTER(ctypes.c_int64), ctypes.c_size_t]
        lib.axon_start_nrt_profile.restype = ctypes.c_int64
        lib.axon_stop_nrt_profile.argtypes = [ctypes.c_char_p]
        lib.axon_stop_nrt_profile.restype = ctypes.c_int64

        @contextlib.contextmanager
        def _hookfn(output_dir, device_ids):
            import jax
            jax.devices()
            if device_ids:
                ids = (ctypes.c_int64 * len(device_ids))(*device_ids)
                rc = lib.axon_start_nrt_profile(ids, len(device_ids))
            else:
                rc = lib.axon_start_nrt_profile(None, 0)
            if rc != 0:
                raise RuntimeError(f"axon_start_nrt_profile rc={rc}")
            try:
                yield
            finally:
                n = lib.axon_stop_nrt_profile(str(output_dir).encode())
                print(f"profile: {n} file(s) written to {output_dir}")

        mod.set_axon_ntff_profile_hook(_hookfn)
    except OSError:
        pass


def _build_program():
    from contextlib import ExitStack

    import concourse.tile as tile
    from concourse import bacc, mybir

    F32 = mybir.dt.float32
    BF16 = mybir.dt.bfloat16
    AF = mybir.ActivationFunctionType
    ALU = mybir.AluOpType

    nc = bacc.Bacc("TRN2", target_bir_lowering=False, debug=False,
                   num_devices=8)

    # all tensor inputs arrive pre-arranged in SBUF layout [128, k, n]
    # (host does the transpose) so every DMA is long contiguous runs
    xT_d = nc.dram_tensor("xT", [P, KT * T], BF16, kind="ExternalInput").ap()
    wq_d = nc.dram_tensor("wq", [P, KT * QD], BF16, kind="ExternalInput").ap()
    wk_d = nc.dram_tensor("wk", [P, KT * QD], BF16, kind="ExternalInput").ap()
    wv_d = nc.dram_tensor("wv", [P, KT * QD], BF16, kind="ExternalInput").ap()
    wo_d = nc.dram_tensor("wo", [P, MT * D], BF16, kind="ExternalInput").ap()
    bq_d = nc.dram_tensor("bq", [P, MT], F32, kind="ExternalInput").ap()
    bk_d = nc.dram_tensor("bk", [P, MT], F32, kind="ExternalInput").ap()
    bv_d = nc.dram_tensor("bv", [1, QD], BF16, kind="ExternalInput").ap()
    msk_d = nc.dram_tensor("msk", [P, P], BF16, kind="ExternalInput").ap()
    ones_d = nc.dram_tensor("ones", [P, P], BF16, kind="ExternalInput").ap()
    out_d = nc.dram_tensor("out", [T, D], F32, kind="ExternalOutput").ap()

    xT_k = xT_d.rearrange("p (k t) -> p k t", k=KT)      # [128, 8, 2048]
    wq_k = wq_d.rearrange("p (k m) -> p k m", k=KT)      # [128, 8, 512]
    wk_k = wk_d.rearrange("p (k m) -> p k m", k=KT)
    wv_k = wv_d.rearrange("p (k m) -> p k m", k=KT)
    wo_k = wo_d.rearrange("p (k e) -> p k e", k=MT)      # [128, 4, 1024]

    with tile.TileContext(nc) as tc, ExitStack() as ctx:
        persist = ctx.enter_context(tc.tile_pool(name="persist", bufs=1))

        qt = [persist.tile([P, T], BF16, name=f"qt{i}") for i in range(MT)]
        kt_ = [persist.tile([P, T], BF16, name=f"kt{i}") for i in range(MT)]
        v3 = [persist.tile([P, NH, HD + 1], BF16, name=f"v3_{i}")
              for i in range(TT)]
        at = [persist.tile([P, T], BF16, name=f"at{i}") for i in range(MT)]
        xt_all = persist.tile([P, KT, T], BF16, name="xt")

        wq_sb = persist.tile([P, KT, QD], BF16, name="wq")
        wk_sb = persist.tile([P, KT, QD], BF16, name="wk")
        bq_sb = persist.tile([P, MT], F32, name="bq")
        bk_sb = persist.tile([P, MT], F32, name="bk")
        bv_sb = persist.tile([1, QD], BF16, name="bv")
        onesrow = persist.tile([1, P], BF16, name="onesrow")
        tri_sb = persist.tile([P, P], BF16, name="tri")
        wv_sb = persist.tile([P, KT, QD], BF16, name="wv")
        wo_sb = persist.tile([P, MT, D], BF16, name="wo")

        nc.sync.dma_start(wq_sb[:], wq_k)
        nc.sync.dma_start(xt_all[:, :, 0:T // 2], xT_k[:, :, 0:T // 2])
        nc.sync.dma_start(wk_sb[:], wk_d.rearrange("p (k m) -> p k m", k=KT))
        nc.sync.dma_start(bq_sb[:], bq_d)
        nc.sync.dma_start(bk_sb[:], bk_d)
        nc.sync.dma_start(xt_all[:, :, T // 2:T], xT_k[:, :, T // 2:T])
        nc.sync.dma_start(bv_sb[:], bv_d)
        nc.sync.dma_start(onesrow[:], ones_d[0:1, :])
        nc.sync.dma_start(tri_sb[:], msk_d)
        nc.sync.dma_start(wv_sb[:], wv_k)
        nc.sync.dma_start(wo_sb[:], wo_k)
        for tt in range(TT):
            nc.vector.memset(v3[tt][:, :, HD:HD + 1], 1.0)

        # ---- phase A1: Q^T, K^T projections --------------------------------
        with tc.tile_pool(name="pjps1", bufs=1, space="PSUM") as pjp:
            # chunk-inner so each weight tile is loaded into the PE once
            # and reused for all four 512-column chunks
            for mt in range(MT):
                for w_sb, dst, b_sb in ((wq_sb, qt, bq_sb), (wk_sb, kt_, bk_sb)):
                    ps = [pjp.tile([P, ACH], F32, name="pj", bufs=8)
                          for _ in range(NACH)]
                    for k in range(KT):
                        for nch in range(NACH):
                            nc.tensor.matmul(
                                ps[nch][:], w_sb[:, k, mt * P:(mt + 1) * P],
                                xt_all[:, k, nch * ACH:(nch + 1) * ACH],
                                start=(k == 0), stop=(k == KT - 1))
                    for nch in range(NACH):
                        csl = slice(nch * ACH, (nch + 1) * ACH)
                        nc.vector.tensor_scalar_add(dst[mt][:, csl],
                                                    ps[nch][:],
                                                    b_sb[:, mt:mt + 1])

        # ---- phases A2/B/C interleaved per query chunk ---------------------
        with tc.tile_pool(name="attnsb", bufs=1) as ap_, \
             tc.tile_pool(name="obp", bufs=3) as obp, \
             tc.tile_pool(name="attnps", bufs=1, space="PSUM") as sp:

            def emit_v_tile(tt):
                psv = sp.tile([P, QD], F32, name="misc", bufs=1)
                for k in range(KT):
                    nc.tensor.matmul(
                        psv[:], xt_all[:, k, tt * P:(tt + 1) * P],
                        wv_sb[:, k, :], start=(k == 0), stop=False)
                nc.tensor.matmul(psv[:], onesrow[0:1, :], bv_sb[0:1, :],
                                 start=False, stop=True)
                nc.vector.tensor_copy(
                    v3[tt][:, :, 0:HD],
                    psv[:].rearrange("p (h d) -> p h d", d=HD))

            def emit_attn_chunk(ic, fillers=()):
                """Attention for query chunk ic, as 4 head-pairs, with the
                PE stream software-pipelined: S of j-tile jt+1 is emitted
                before AV of j-tile jt so exp latency is hidden. One filler
                (a V-tile projection or an out-projection group for another
                chunk) is emitted per head-pair boundary to keep the PE fed
                while the scalar engine works through the exps."""
                isl = slice(ic * ICH, (ic + 1) * ICH)
                njt = 4 * ic + 4
                fillers = list(fillers)
                pending = []
                for hp in range(MT):
                    opsA = sp.tile([HD + 1, ICH], F32, name="opsum", bufs=3)
                    opsB = sp.tile([HD + 1, ICH], F32, name="opsum", bufs=3)
                    s2s, e2s = {}, {}

                    def emit_s(jt):
                        s2 = sp.tile([P, 2 * ICH], F32, name="spsum", bufs=2)
                        jsl = slice(jt * P, (jt + 1) * P)
                        nc.tensor.matmul(s2[:, 0:ICH], kt_[hp][0:HD, jsl],
                                         qt[hp][0:HD, isl],
                                         start=True, stop=True)
                        nc.tensor.matmul(s2[:, ICH:2 * ICH],
                                         kt_[hp][HD:P, jsl],
                                         qt[hp][HD:P, isl],
                                         start=True, stop=True)
                        s2s[jt] = s2

                    def emit_exp(jt):
                        e2 = ap_.tile([P, 2 * ICH], BF16, name="e", bufs=3)
                        nc.scalar.activation(e2[:], s2s.pop(jt)[:], AF.Exp)
                        kdiag = jt - 4 * ic
                        if kdiag >= 0:
                            # zero the diagonal block's upper triangle
                            c0 = kdiag * P
                            for half in range(2):
                                o = half * ICH + c0
                                nc.vector.tensor_tensor(
                                    e2[:, o:o + P], e2[:, o:o + P],
                                    tri_sb[:], op=ALU.mult)
                        e2s[jt] = e2

                    def emit_av(jt):
                        # columns left of the diagonal block are causally
                        # invalid — restrict the accumulation to the valid
                        # column range instead of zeroing them
                        kdiag = jt - 4 * ic
                        c0 = max(kdiag, 0) * P
                        e2 = e2s.pop(jt)
                        nc.tensor.matmul(opsA[:, c0:], v3[jt][:, 2 * hp, :],
                                         e2[:, c0:ICH],
                                         start=(jt == 0),
                                         stop=(jt == njt - 1))
                        nc.tensor.matmul(opsB[:, c0:],
                                         v3[jt][:, 2 * hp + 1, :],
                                         e2[:, ICH + c0:2 * ICH],
                                         start=(jt == 0),
                                         stop=(jt == njt - 1))

                    emit_s(0)
                    for jt in range(1, njt):
                        emit_s(jt)
                        if jt == 2 and pending:
                            pending.pop()()
                        emit_exp(jt - 1)
                        emit_av(jt - 1)
                    emit_exp(njt - 1)
                    emit_av(njt - 1)

                    if ic == 0 and pending:
                        pending.pop()()

                    def normalize(hp=hp, opsA=opsA, opsB=opsB):
                        # normalize straight out of PSUM: in0 is PSUM so the
                        # SBUF base-partition pairing rule doesn't apply
                        for half, ops in ((0, opsA), (1, opsB)):
                            po = half * HD
                            dn = ap_.tile([1, ICH], F32, name="dn", bufs=4)
                            nc.vector.tensor_copy(dn[:], ops[HD:HD + 1, :])
                            recip = ap_.tile([1, ICH], F32, name="recip",
                                             bufs=4)
                            nc.vector.reciprocal_approx_fast(recip[:], dn[:])
                            rb = ap_.tile([HD, ICH], F32, name="rb", bufs=4)
                            nc.gpsimd.partition_broadcast(rb[:], recip[:])
                            nc.vector.tensor_tensor(
                                at[hp][po:po + HD, isl], ops[0:HD, :], rb[:],
                                op=ALU.mult)

                    pending.append(normalize)
                    if fillers:
                        fillers.pop(0)()

                while pending:
                    pending.pop()()
                for f in fillers:
                    f()

            def emit_out_group(mt, nch2):
                pso = sp.tile([P, 512], F32, name="misc", bufs=1)
                for k in range(MT):
                    nc.tensor.matmul(
                        pso[:], at[k][:, mt * P:(mt + 1) * P],
                        wo_sb[:, k, nch2 * 512:(nch2 + 1) * 512],
                        start=(k == 0), stop=(k == MT - 1))
                ob = obp.tile([P, 512], F32, name="ob")
                nc.vector.tensor_copy(ob[:], pso[:])
                nc.sync.dma_start(
                    out_d[mt * P:(mt + 1) * P,
                          nch2 * 512:(nch2 + 1) * 512], ob[:])

            for tt in range(4):
                emit_v_tile(tt)
            for ic in range(NIC):
                fillers = []
                if ic + 1 < NIC:
                    fillers += [
                        (lambda tt=tt: emit_v_tile(tt))
                        for tt in range(4 * ic + 4, 4 * ic + 8)]
                if ic > 0:
                    fillers += [
                        (lambda mt=mt, n=n: emit_out_group(mt, n))
                        for mt in range(4 * (ic - 1), 4 * ic)
                        for n in range(2)]
                emit_attn_chunk(ic, fillers)
            for mt in range(12, 16):
                for n in range(2):
                    emit_out_group(mt, n)

    nc.compile()
    return nc


def _get_program():
    global _PROGRAM
    if _PROGRAM is None:
        _install_ntff_hook()
        _PROGRAM = _build_program()
    return _PROGRAM


def _make_masks():
    """Multiplicative upper-triangle zero mask [128, 128] for the diagonal
    128x128 block of each S^T tile: entry (j, i) = 1 if j <= i else 0."""
    j = np.arange(P)[:, None]
    i = np.arange(P)[None, :]
    return (j <= i).astype(np.float32)


def make_in_maps(x, Wq, bq, Wk, bk, Wv, bv, Wo, bo):
    import ml_dtypes
    bf16 = ml_dtypes.bfloat16

    def sbl(a, k):
        """[k*128, n] -> SBUF layout [128, k*n] (partition-major runs)."""
        n = a.shape[1]
        return np.ascontiguousarray(
            a.reshape(k, P, n).transpose(1, 0, 2).reshape(P, k * n)
        ).astype(bf16)

    masks = _make_masks()
    ones = np.ones((P, P), dtype=np.float32)
    in_maps = []
    for c in range(8):
        b, hg = c // 2, c % 2
        sl = slice(hg * QD, (hg + 1) * QD)
        in_maps.append({
            "xT": sbl(np.ascontiguousarray(x[b].T), KT),
            "wq": sbl(Wq[:, sl] * SCALE, KT),
            "wk": sbl(Wk[:, sl], KT),
            "wv": sbl(Wv[:, sl], KT),
            "wo": sbl(Wo[sl, :], MT),
            "bq": np.ascontiguousarray((bq[sl] * SCALE).reshape(MT, P).T),
            "bk": np.ascontiguousarray(bk[sl].reshape(MT, P).T),
            "bv": bv[sl].reshape(1, QD).astype(bf16),
            "msk": masks.astype(bf16),
            "ones": ones.astype(bf16),
        })
    return in_maps


def run(inputs, trace=False):
    from concourse.bass_utils import run_bass_kernel_spmd

    nc = _get_program()
    in_maps = make_in_maps(**inputs)
    res = run_bass_kernel_spmd(nc, in_maps, list(range(8)), trace=trace)
    bo = inputs["bo"]
    out = np.empty((B, T, D), dtype=np.float32)
    for b in range(B):
        out[b] = res.results[2 * b]["out"] + res.results[2 * b + 1]["out"] + bo
    return out, res


def kernel(**inputs):
    inputs = {k: np.asarray(v) for k, v in inputs.items()}
    out, _ = run(inputs)
    return out
